# revision 10
# baseline (speedup 1.0000x reference)
"""Trainium2 Bass kernel for AVIF adaptive transform.

Computes, per channel c: y = T_c @ P @ T_c^T on each 8x8 block of x,
then g = sigmoid(W2 @ relu(W1 @ y + b1) + b2) (1x1 convs over channels),
returns y * g.

Strategy (8 cores, data parallel over (batch, H-half)):
  Each core gets a slab x[b, :, h0:h0+256, :] of shape [64, 256, 512].

  Phase A (block transform), per (channel, 128-row tile):
    M1: matmul(lhsT=X_chunk[128h,128w], rhs=BD_c) -> X1[w, h']   (vertical mix + transpose)
    M2: matmul(lhsT=X1_chunk[128w,128h'], rhs=BD_c) -> Y[h', w]  (horizontal mix + transpose back)
    where BD_c = blockdiag_16(T_c^T) [128,128].
    Y is written (bf16) to an HBM scratch tensor.

  Phase B (gate + multiply), per 8-row stripe:
    Reload scratch with channels on partitions: R[(hg,c), (hh,w)],
    conv1/conv2 as streaming matmuls with blockdiag_2(W^T) weights,
    bias+relu / bias+sigmoid on ScalarE, y*g on VectorE, DMA out fp32.
"""

import numpy as np
import ml_dtypes

BLOCK = 8
C = 64
B = 4
H = 512
W = 512
NCORES = 8
HL = H * B // NCORES  # 256 rows per core


def _prep_constants(tw, w1, b1, w2, b2):
    """Host-side constant layouts.

    bd_flat  [128, C*128] f16 : bd_flat[p, c*128+q] = BD_c[p, q],
                                 BD_c[8m+i, 8m+p'] = tw[c, p', i]
    wbd_flat [128, 2*128] f16 : blockdiag_2(W^T) for conv1, conv2
    bias_flat[128, 2]     f32  : [tile(b1,2), tile(b2,2)]
    """
    f16 = np.float16
    bd = np.zeros((C, 128, 128), np.float32)
    twT = np.ascontiguousarray(tw.transpose(0, 2, 1))  # [c, i, p']
    for m in range(16):
        bd[:, 8 * m:8 * m + 8, 8 * m:8 * m + 8] = twT
    bd_flat = np.ascontiguousarray(
        bd.transpose(1, 0, 2).reshape(128, C * 128)).astype(f16)

    wbd = np.zeros((2, 128, 128), np.float32)
    for hg in range(2):
        wbd[0, 64 * hg:64 * hg + 64, 64 * hg:64 * hg + 64] = w1.T
        wbd[1, 64 * hg:64 * hg + 64, 64 * hg:64 * hg + 64] = w2.T
    wbd_flat = np.ascontiguousarray(
        wbd.transpose(1, 0, 2).reshape(128, 2 * 128)).astype(f16)

    bias_flat = np.stack([np.tile(b1, 2), np.tile(b2, 2)], axis=1)
    bias_flat = np.ascontiguousarray(bias_flat).astype(np.float32)
    return bd_flat, wbd_flat, bias_flat


def _build_nc(hl=HL, w=W):
    import concourse.mybir as mybir
    from concourse import bacc
    from concourse.tile import TileContext
    from contextlib import ExitStack

    f32 = mybir.dt.float32
    f16 = mybir.dt.float16
    AF = mybir.ActivationFunctionType

    nwk = w // 128   # w chunks per row tile
    nht = hl // 128  # 128-row tiles per slab-channel
    nhb = hl // 8    # 8-row stripes in phase B

    nc = bacc.Bacc(None)
    xs = nc.declare_dram_parameter("xs", [C, hl, w], f32, isOutput=False)
    bdp = nc.declare_dram_parameter("bd", [128, C * 128], f16, isOutput=False)
    wbdp = nc.declare_dram_parameter("wbd", [128, 2 * 128], f16, isOutput=False)
    biap = nc.declare_dram_parameter("bia", [128, 2], f32, isOutput=False)
    out = nc.declare_dram_parameter("out", [C, hl, w], f32, isOutput=True)
    scr = nc.dram_tensor("scr", [C, hl, w], f16)

    with TileContext(nc) as tc, ExitStack() as ctx:
        const = ctx.enter_context(tc.tile_pool(name="const", bufs=1))
        bd_t = const.tile([128, C * 128], f16)
        nc.sync.dma_start(out=bd_t[:], in_=bdp[:])
        wbd_t = const.tile([128, 2 * 128], f16)
        nc.sync.dma_start(out=wbd_t[:], in_=wbdp[:])
        bia_t = const.tile([128, 2], f32)
        nc.sync.dma_start(out=bia_t[:], in_=biap[:])

        pXF = ctx.enter_context(tc.tile_pool(name="pXF", bufs=3))
        pX = ctx.enter_context(tc.tile_pool(name="pX", bufs=4))
        pX1 = ctx.enter_context(tc.tile_pool(name="pX1", bufs=3))
        pY = ctx.enter_context(tc.tile_pool(name="pY", bufs=3))
        pA = ctx.enter_context(tc.tile_pool(name="pA", bufs=2, space="PSUM"))
        pB = ctx.enter_context(tc.tile_pool(name="pB", bufs=2, space="PSUM"))

        # ---- Phase A: per-channel 8x8 block transform ----
        for c in range(C):
            bdc = bd_t[:, c * 128:(c + 1) * 128]
            for ht in range(nht):
                xf = pXF.tile([128, w], f32)
                nc.sync.dma_start(
                    out=xf[:], in_=xs[c, ht * 128:(ht + 1) * 128, :])
                xt = pX.tile([128, w], f16)
                nc.gpsimd.tensor_copy(xt[:], xf[:])
                pa = pA.tile([128, w], f32)
                for wk in range(nwk):
                    nc.tensor.matmul(
                        pa[:, wk * 128:(wk + 1) * 128],
                        lhsT=xt[:, wk * 128:(wk + 1) * 128],
                        rhs=bdc, start=True, stop=True)
                x1 = pX1.tile([128, w], f16)
                nc.scalar.activation(x1[:], pa[:], AF.Copy)
                pb = pB.tile([128, w], f32)
                for wk in range(nwk):
                    nc.tensor.matmul(
                        pb[:, wk * 128:(wk + 1) * 128],
                        lhsT=x1[:, wk * 128:(wk + 1) * 128],
                        rhs=bdc, start=True, stop=True)
                yt = pY.tile([128, w], f16)
                nc.vector.tensor_copy(yt[:], pb[:])
                nc.sync.dma_start(
                    out=scr[c, ht * 128:(ht + 1) * 128, :], in_=yt[:])

        # ---- Phase B: 1x1 conv gate + multiply ----
        pR = ctx.enter_context(tc.tile_pool(name="pR", bufs=3))
        pG1 = ctx.enter_context(tc.tile_pool(name="pG1", bufs=2))
        pG2 = ctx.enter_context(tc.tile_pool(name="pG2", bufs=2))
        pO = ctx.enter_context(tc.tile_pool(name="pO", bufs=3))
        pC = ctx.enter_context(tc.tile_pool(name="pC", bufs=2, space="PSUM"))
        pD = ctx.enter_context(tc.tile_pool(name="pD", bufs=2, space="PSUM"))

        nj = (4 * w) // 512  # 512-px conv chunks per stripe

        for hb in range(nhb):
            r0 = hb * 8
            rt = pR.tile([128, 4 * w], f16)
            nc.sync.dma_start(out=rt[0:64, :], in_=scr[:, r0:r0 + 4, :])
            nc.sync.dma_start(out=rt[64:128, :], in_=scr[:, r0 + 4:r0 + 8, :])
            ot = pO.tile([128, 4 * w], f32)
            for j in range(nj):
                rj = rt[:, j * 512:(j + 1) * 512]
                pc = pC.tile([128, 512], f32)
                nc.tensor.matmul(pc[:], lhsT=wbd_t[:, 0:128], rhs=rj,
                                 start=True, stop=True)
                g1 = pG1.tile([128, 512], f16)
                nc.scalar.activation(g1[:], pc[:], AF.Relu,
                                     bias=bia_t[:, 0:1])
                pd = pD.tile([128, 512], f32)
                nc.tensor.matmul(pd[:], lhsT=wbd_t[:, 128:256], rhs=g1[:],
                                 start=True, stop=True)
                g2 = pG2.tile([128, 512], f16)
                nc.scalar.activation(g2[:], pd[:], AF.Sigmoid,
                                     bias=bia_t[:, 1:2])
                nc.vector.tensor_mul(ot[:, j * 512:(j + 1) * 512], g2[:], rj)
            nc.sync.dma_start(out=out[:, r0:r0 + 4, :], in_=ot[0:64, :])
            nc.sync.dma_start(out=out[:, r0 + 4:r0 + 8, :], in_=ot[64:128, :])

    return nc


_NC_CACHE = {}


def _get_nc(hl=HL, w=W):
    key = (hl, w)
    if key not in _NC_CACHE:
        nc = _build_nc(hl, w)
        nc.finalize()
        _NC_CACHE[key] = nc
    return _NC_CACHE[key]


def _make_in_maps(x, tw, w1, b1, w2, b2):
    x = np.asarray(x, np.float32)
    bd_flat, wbd_flat, bias_flat = _prep_constants(
        np.asarray(tw, np.float32), np.asarray(w1, np.float32),
        np.asarray(b1, np.float32), np.asarray(w2, np.float32),
        np.asarray(b2, np.float32))
    in_maps = []
    for k in range(NCORES):
        b, half = divmod(k, NCORES // B)
        xs = np.ascontiguousarray(x[b, :, half * HL:(half + 1) * HL, :])
        in_maps.append({"xs": xs, "bd": bd_flat, "wbd": wbd_flat,
                        "bia": bias_flat})
    return in_maps


def _assemble(results):
    outf = np.empty((B, C, H, W), np.float32)
    for k in range(NCORES):
        b, half = divmod(k, NCORES // B)
        outf[b, :, half * HL:(half + 1) * HL, :] = results[k]["out"]
    return outf


def kernel(x, tw, w1, b1, w2, b2):
    from concourse import bass2jax

    nc = _get_nc()
    in_maps = _make_in_maps(x, tw, w1, b1, w2, b2)
    results = bass2jax.run_bass_via_pjrt(nc, in_maps, n_cores=NCORES)
    return _assemble(results)


def make_bench(x, tw, w1, b1, w2, b2, nc=None):
    """Build a reusable device-resident runner for timing.

    Returns (run, get_output): run() executes the SPMD kernel once on
    device-held buffers and blocks; get_output() fetches the assembled
    full output for a correctness check.
    """
    import jax
    from jax.sharding import Mesh, PartitionSpec
    from jax.experimental.shard_map import shard_map
    from concourse import bass2jax
    import concourse.mybir as mybir

    bass2jax.install_neuronx_cc_hook()
    if nc is None:
        nc = _get_nc()
    in_maps = _make_in_maps(x, tw, w1, b1, w2, b2)

    partition_name = (nc.partition_id_tensor.name
                      if nc.partition_id_tensor else None)
    in_names, out_names, out_avals = [], [], []
    for alloc in nc.m.functions[0].allocations:
        if not isinstance(alloc, mybir.MemoryLocationSet):
            continue
        name = alloc.memorylocations[0].name
        if alloc.kind == "ExternalInput":
            if name != partition_name:
                in_names.append(name)
        elif alloc.kind == "ExternalOutput":
            out_names.append(name)
            out_avals.append(jax.core.ShapedArray(
                tuple(alloc.tensor_shape), mybir.dt.np(alloc.dtype)))
    n_params = len(in_names)
    all_names = in_names + out_names
    if partition_name is not None:
        all_names = all_names + [partition_name]

    def _body(*args):
        operands = list(args)
        if partition_name is not None:
            operands.append(bass2jax.partition_id_tensor())
        outs = bass2jax._bass_exec_p.bind(
            *operands,
            out_avals=tuple(out_avals),
            in_names=tuple(all_names),
            out_names=tuple(out_names),
            lowering_input_output_aliases=(),
            sim_require_finite=True,
            sim_require_nnan=True,
            nc=nc,
        )
        return tuple(outs)

    devices = jax.devices()[:NCORES]
    mesh = Mesh(np.asarray(devices), ("core",))
    n_out = len(out_names)
    sharded = jax.jit(shard_map(
        _body, mesh=mesh,
        in_specs=(PartitionSpec("core"),) * (n_params + n_out),
        out_specs=(PartitionSpec("core"),) * n_out,
        check_rep=False), keep_unused=True)

    concat_in = [
        np.concatenate([np.asarray(in_maps[c][nm]) for c in range(NCORES)],
                       axis=0) for nm in in_names]
    concat_zeros = [
        np.zeros((NCORES * a.shape[0], *a.shape[1:]), a.dtype)
        for a in out_avals]
    sharding = jax.sharding.NamedSharding(mesh, PartitionSpec("core"))
    dev_in = [jax.device_put(a, sharding) for a in concat_in + concat_zeros]

    state = {}

    def run():
        out = sharded(*dev_in)
        jax.block_until_ready(out)
        state["out"] = out
        return out

    def get_output():
        out_arrs = state["out"]
        results = [
            {nm: np.asarray(out_arrs[i]).reshape(
                NCORES, *out_avals[i].shape)[c]
             for i, nm in enumerate(out_names)}
            for c in range(NCORES)]
        return _assemble(results)

    return run, get_output
